# revision 63
# baseline (speedup 1.0000x reference)
"""Multi-head attention (ReLU-gated projections) on 8 Trainium2 NeuronCores.

Problem (hardcoded): B=4, S=1024, H=1024, NH=16, DH=64.
  qp = relu(q @ Wq.T + bq); kp, vp likewise
  alpha = softmax(qh @ kh.T / sqrt(DH)) * mask[q]
  out = (alpha @ vh).reshape(B,S,H) + query

Sharding: 8 cores = 4 batches x 2 head-groups (8 heads / 512 hidden cols each).

fp8 design: all matmuls in fp8 e4m3 (TRN2 flavor: with-inf, max finite 240).
Weights pre-scaled by 32 on the host so their N(0, 1/32) values use e4m3's
normal range; the 32x factors ride through the linear pipeline (qp,kp,vp
all carry 32x) and are compensated in the exp scale (1/(8*32^2)) and a
final /32 on the host. exp also subtracts 3.0 (cancels in softmax) to
keep pt under the 240 cap. Projections and AV use MatmulPerfMode.DoubleRow
(K=256 per instruction, 2x PE throughput); alpha matmuls are
output-rate-bound so they stay plain fp8 with the kz zero-padded-K trick.
The AV stationary keeps a ones column (M=65) so row 64 accumulates sumexp
for free; the per-head V slot is padded to 68 bytes so DoubleRow weight
APs stay 4-byte aligned (ISA restriction s3_lw_dual_fp8).

Host pre-arranges x/w into the exact SBUF layouts so every input DMA is
128 partitions x 4KB contiguous. Consts load first so the PE-clock warmup
(and the ACT exp-table preload) start immediately.

Per-core device kernel (transposed "hidden-on-partitions" layout):
  stage 1: qpT[o,s], kpT[o,s] (transposed) and vp[s,o] (normal) projections
           with fused bias+relu, evacuated to fp8.
  stage 2: per head: alphaT[k,q] psum tiles; pt = exp(alpha*sc - 3) in fp8
           written into paired [128, 2048] tiles; AV via DoubleRow with the
           ones column -> unnormalized hidT (64,S) + sumexp (S) per head;
           host divides, applies mask, adds residual.
"""
import sys

sys.path.insert(0, "/opt/trn_rl_repo")

import os
import numpy as np
import ml_dtypes

import concourse.bass as bass
import concourse.tile as tile
from concourse import bacc, mybir
from concourse import bass_utils

B, S, H = 4, 1024, 1024
NH, DH = 16, 64
NCORES = 8
GROUPS = 2          # head-groups (tensor-parallel dim)
HL = NH // GROUPS   # heads per core = 8
GH = H // GROUPS    # hidden cols per core = 512
KT = H // 128       # contraction k-tiles = 8
OT = GH // 128      # output o-tiles per core = 4
SCALE = 1.0 / float(np.sqrt(DH))
ESC = 32.0          # fp8 weight pre-scale (TRN2 fp8e4 = e4m3-with-inf,
                    # max finite 240: keep relu'd projections under ~170)
VW8 = HL * 68       # padded per-head v slot (64 v + 1 ones + 3 pad) = 544

MODE = os.environ.get("BASS_MM_DT", "fp8")

F32 = mybir.dt.float32
BF16 = mybir.dt.bfloat16
FP8 = mybir.dt.float8e4
DR = mybir.MatmulPerfMode.DoubleRow
E4 = ml_dtypes.float8_e4m3   # e4m3 WITH inf (max 240) — matches TRN2 hw


def build_fp8():
    nc = bacc.Bacc("TRN2", target_bir_lowering=False, debug=False,
                   num_devices=NCORES)

    # x/w arrive pre-arranged in SBUF layout: [128, KT*512] per s-chunk
    x_d = {(w, sc): nc.dram_tensor(f"x{w}{sc}", [128, KT * 512], FP8,
                                   kind="ExternalInput").ap()
           for w in "qkv" for sc in range(2)}
    w_d = {w: nc.dram_tensor(f"w{w}", [128, KT * GH], FP8,
                             kind="ExternalInput").ap()
           for w in "qkv"}
    bqk_d = nc.dram_tensor("bqk", [128, 2 * OT], F32, kind="ExternalInput").ap()
    bv_d = nc.dram_tensor("bv", [1, GH], FP8, kind="ExternalInput").ap()
    ones_d = nc.dram_tensor("onesd", [128, 128], FP8,
                            kind="ExternalInput").ap()
    hid_d = nc.dram_tensor("hid", [HL * (DH + 1), S], F32,
                           kind="ExternalOutput").ap()

    EXP_SCALE = SCALE / (ESC * ESC)
    EXP_BIAS = -3.0   # pt = exp(alpha/8 - 3): keeps exp under e4m3 max 240;
                      # cancels in hid/sumexp

    with tile.TileContext(nc) as tc:
        with tc.tile_pool(name="sb", bufs=1) as sb, \
             tc.tile_pool(name="ps", bufs=1, space="PSUM") as ps:

            # ---- persistent tiles ----
            wq_t = sb.tile([128, KT * GH], FP8, tag="wq", name="wq")
            wk_t = sb.tile([128, KT * GH], FP8, tag="wk", name="wk")
            wv_t = sb.tile([128, KT * GH], FP8, tag="wv", name="wv")
            qp_t = [sb.tile([128, S], FP8, tag=f"qp{t}", name=f"qp{t}")
                    for t in range(OT)]
            kz_t = [[sb.tile([128, S], FP8, tag=f"kz{t}{h}",
                             name=f"kz{t}{h}") for h in range(2)]
                    for t in range(OT)]
            kz_zeroed = set()
            vp_t = sb.tile([128, KT * VW8], FP8, tag="vp", name="vp")
            bqk_t = sb.tile([128, 2 * OT], F32, tag="bqk", name="bqk")
            bv_t = sb.tile([1, GH], FP8, tag="bv", name="bv")
            ones_t = sb.tile([1, 128], FP8, tag="ones", name="ones")
            ones64_t = sb.tile([128, KT * HL], FP8, tag="ones64",
                               name="ones64")
            expb_t = sb.tile([128, 1], F32, tag="expb", name="expb")
            nc.vector.memset(expb_t[:], EXP_BIAS)

            # ---- warmup from memset tiles: no DMA dependency, so the PE
            #      clock ramp and the ACT exp-table preload start at ~1us ----
            wstat = sb.tile([1, 128], FP8, tag="wstat", name="wstat")
            wmov = sb.tile([1, 512], FP8, tag="wmov", name="wmov")
            nc.vector.memset(wstat[:], 1.0)
            nc.vector.memset(wmov[:], 1.0)
            warm = ps.tile([65, 512], F32, tag="av", bufs=2, name="warm")
            for i in range(12):
                nc.tensor.matmul(warm[:], wstat[:, 0:65], wmov[:],
                                 start=True, stop=True)
            dummy_exp = sb.tile([1, 8], F32, tag="dummy_exp", name="dummy_exp")
            nc.scalar.activation(dummy_exp[:], wmov[0:1, 0:8],
                                 mybir.ActivationFunctionType.Exp, scale=1.0)

            # ---- loads: whole tiles (4KB contiguous runs), three rings in
            #      parallel, priority-ordered by first use ----
            x_t = {}
            rings = [nc.sync, nc.scalar, nc.gpsimd]
            ring_i = [0]

            def x_ld(which, sc, eng):
                t = sb.tile([128, KT * 512], FP8, tag=f"x{which}{sc}",
                            name=f"x{which}_{sc}")
                x_t[(which, sc)] = t
                eng.dma_start(t[:], x_d[(which, sc)])

            def x3(which, sc):
                return x_t[(which, sc)][:].rearrange("p (k s) -> p k s", s=512)

            nc.sync.dma_start(bqk_t[:], bqk_d)
            x_ld("q", 0, nc.sync)
            x_ld("k", 0, nc.scalar)
            nc.gpsimd.dma_start(wq_t[:], w_d["q"])
            nc.gpsimd.dma_start(wk_t[:], w_d["k"])
            x_ld("k", 1, nc.sync)
            x_ld("q", 1, nc.scalar)
            nc.gpsimd.dma_start(bv_t[:], bv_d)
            nc.gpsimd.dma_start(ones_t[:], ones_d[0:1, :])
            nc.sync.dma_start(ones64_t[:], ones_d[:, 0:KT * HL])
            x_ld("v", 0, nc.sync)
            x_ld("v", 1, nc.scalar)
            nc.gpsimd.dma_start(wv_t[:], w_d["v"])

            # ones column of the AV stationary
            v4 = vp_t[:].rearrange("p (k n c) -> p k n c", n=HL, c=68)
            nc.vector.tensor_copy(
                v4[:, :, :, DH:DH + 1],
                ones64_t[:].rearrange("p (k n one) -> p k n one", n=HL, one=1))

            pp_live = {}

            def proj_qk(sc, ot, which, part=None):
                """one o-tile, one s-chunk of the transposed q/k projection;
                part 0/1 emit half the DR chain each (fill-unit sizing),
                part None emits the whole group."""
                w_t = wq_t if which == "q" else wk_t
                w3 = w_t[:].rearrange("p (k o) -> p k o", o=GH)
                xv_ = x3(which, sc)
                if part != 1:
                    pp_live[(sc, ot, which)] = ps.tile(
                        [128, 1024], F32, tag="alpha", bufs=3,
                        name=f"pp{which}_{sc}_{ot}")
                pp = pp_live[(sc, ot, which)]
                kps = range(KT // 2) if part is None else (
                    range(2) if part == 0 else range(2, 4))
                for kp in kps:
                    nc.tensor.matmul(
                        pp[:, 0:512],
                        w3[:, 2 * kp:2 * kp + 2, ot * 128:(ot + 1) * 128],
                        xv_[:, 2 * kp:2 * kp + 2, :],
                        start=(kp == 0), stop=(kp == KT // 2 - 1),
                        perf_mode=DR)
                if part == 0:
                    return
                wi = 0 if which == "q" else 1
                bias = bqk_t[:, wi * OT + ot:wi * OT + ot + 1]
                ssl = slice(sc * 512, (sc + 1) * 512)
                if which == "q":
                    if (sc, ot) == (1, 0):
                        # critical-path evac before the first alpha: ACT is
                        # idle pre-stream while the DVE queue is backed up
                        nc.scalar.activation(
                            qp_t[ot][:, ssl], pp[:, 0:512],
                            mybir.ActivationFunctionType.Relu,
                            bias=bias, scale=1.0)
                    else:
                        nc.vector.tensor_scalar(
                            qp_t[ot][:, ssl], pp[:, 0:512], bias, 0.0,
                            mybir.AluOpType.add, mybir.AluOpType.max)
                else:
                    for h in range(2):
                        pr = slice(h * 64, h * 64 + 64)
                        if (sc, ot) == (0, 0):
                            # pre-stream critical path: ACT idles until the
                            # first exp while the DVE queue is backed up
                            nc.scalar.activation(
                                kz_t[ot][h][pr, ssl], pp[pr, 0:512],
                                mybir.ActivationFunctionType.Relu,
                                bias=bias[pr, :], scale=1.0)
                        else:
                            nc.vector.tensor_scalar(
                                kz_t[ot][h][pr, ssl], pp[pr, 0:512],
                                bias[pr, :], 0.0,
                                mybir.AluOpType.add, mybir.AluOpType.max)
                pp_live.pop((sc, ot, which))

            def proj_v(sc, j, part=None):
                """one s-tile (128 rows of vp) within chunk sc"""
                st = sc * 4 + j
                wv3 = wv_t[:].rearrange("p (k o) -> p k o", o=GH)
                xv_ = x3("v", sc)
                if part != 1:
                    pp_live[("v", st)] = ps.tile([128, 1024], F32,
                                                 tag="alpha", bufs=3,
                                                 name=f"ppv_{st}")
                    nc.tensor.matmul(pp_live[("v", st)][:, 0:512],
                                     ones_t[:], bv_t[:],
                                     start=True, stop=False)
                pp = pp_live[("v", st)]
                kps = range(KT // 2) if part is None else (
                    range(2) if part == 0 else range(2, 4))
                for kp in kps:
                    nc.tensor.matmul(
                        pp[:, 0:512],
                        xv_[:, 2 * kp:2 * kp + 2, j * 128:(j + 1) * 128],
                        wv3[:, 2 * kp:2 * kp + 2, :],
                        start=False, stop=(kp == KT // 2 - 1),
                        perf_mode=DR)
                if part == 0:
                    return
                v3 = vp_t[:, st * VW8:(st + 1) * VW8].rearrange(
                    "p (n c) -> p n c", c=68)
                p3 = pp[:, 0:512].rearrange("p (n c) -> p n c", c=DH)
                nc.vector.tensor_scalar(
                    v3[:, :, 0:DH], p3, 0.0, None, mybir.AluOpType.max)
                pp_live.pop(("v", st))

            pt_all = {}
            fill_q = []

            def alphas(n0, pops=(1, 3, 5, 7)):
                """alpha + exp for head pair (n0, n0+1), head-major so each
                head's pt tiles complete early and its AV can start while the
                other head's exps still stream.  pt tiles are paired
                [128, 2048] (two k-tiles) so AV consumes them via DoubleRow.
                Between apt tiles, pop small PE work units from fill_q so the
                PE's ACT-rate-limited stall time does useful work."""
                t = n0 // 2
                if t not in kz_zeroed:
                    kz_zeroed.add(t)
                    nc.gpsimd.memset(kz_t[t][0][64:128, :], 0.0)
                    nc.gpsimd.memset(kz_t[t][1][0:64, :], 0.0)
                for h in range(2):
                    pts = []
                    cur = None
                    for k in range(KT):
                        apt = ps.tile([128, 1024], F32, tag="alpha", bufs=3,
                                      name=f"alp_{n0 + h}_{k}")
                        for qc in range(2):
                            nc.tensor.matmul(
                                apt[:, qc * 512:(qc + 1) * 512],
                                kz_t[t][h][:, k * 128:(k + 1) * 128],
                                qp_t[t][:, qc * 512:(qc + 1) * 512],
                                start=True, stop=True)
                        half = k % 2
                        if half == 0:
                            cur = sb.tile([128, 2048], FP8, tag="pt",
                                          bufs=32, name=f"pt_{n0 + h}_{k}")
                            pts.append(cur)
                        nc.scalar.activation(
                            cur[:, half * 1024:(half + 1) * 1024], apt[:],
                            mybir.ActivationFunctionType.Exp, scale=EXP_SCALE,
                            bias=expb_t[:])
                        if k in pops and fill_q:
                            fill_q.pop(0)()
                    pt_all[n0 + h] = pts

            hid_tiles = {}
            av_live = {}

            def avs_qc(n, qc, last=False, part=None):
                pts = pt_all[n]
                if qc == 0 and part != 1:
                    hid_tiles[n] = sb.tile([DH + 1, S], F32, tag="hid",
                                           bufs=3, name=f"hid_{n}")
                hid_t = hid_tiles[n]
                if part != 1:
                    av_live[(n, qc)] = ps.tile([DH + 1, 512], F32, tag="av",
                                               bufs=2, name=f"av_{n}_{qc}")
                av = av_live[(n, qc)]
                kps = range(KT // 2) if part is None else (
                    range(2) if part == 0 else range(2, 4))
                for kp in kps:
                    nc.tensor.matmul(
                        av[:],
                        v4[:, 2 * kp:2 * kp + 2, n, 0:DH + 1],
                        pts[kp][:].rearrange(
                            "p (k s) -> p k s",
                            s=1024)[:, :, qc * 512:(qc + 1) * 512],
                        start=(kp == 0), stop=(kp == KT // 2 - 1),
                        perf_mode=DR)
                if part == 0:
                    return
                av_live.pop((n, qc))
                if last:
                    # ACT is idle after its final exp — use it so the two
                    # tail evacuations run on different engines
                    nc.scalar.copy(
                        hid_t[:, qc * 512:(qc + 1) * 512], av[:])
                else:
                    nc.vector.tensor_copy(
                        hid_t[:, qc * 512:(qc + 1) * 512], av[:])
                # never the scalar ring: a DMA descriptor op there would
                # steal ~0.8us from the ACT exp stream
                eng = nc.sync if ring_i[0] % 2 == 0 else nc.gpsimd
                ring_i[0] += 1
                eng.dma_start(
                    hid_d[n * (DH + 1):(n + 1) * (DH + 1),
                          qc * 512:(qc + 1) * 512],
                    hid_t[:, qc * 512:(qc + 1) * 512])
                if qc == 1:
                    pt_all.pop(n)
                    hid_tiles.pop(n)

            # ---- emission schedule: the exp stream (ACT) is the metronome.
            #      All other PE work is queued as fill units popped between
            #      alpha tiles, so the PE's ACT-limited stalls do the
            #      projections and AV chunks. Queue order respects deps:
            #      o-tile t's projections drain inside alphas(2(t-1)). ----
            def u2(f, *a):
                fill_q.append(lambda: f(*a, part=0))
                fill_q.append(lambda: f(*a, part=1))

            # alphas(0) k-tiles 0-3 read only the sc0 half of kz o-tile 0, so
            # the sc1 k-projection rides the fill queue (its parts pop at
            # k=1,2 — done before the k=4 alpha tile needs them)
            proj_qk(0, 0, "q")
            # o-tile-1's sc0 q-projection needs only first-wave DMA data
            # (xq0+wq): run it in the PE idle slot while wk/xq1 land
            proj_qk(0, 1, "q")
            proj_qk(0, 0, "k")
            proj_qk(1, 0, "q")
            u2(proj_qk, 1, 0, "k")
            u2(proj_qk, 0, 1, "k")
            u2(proj_qk, 1, 1, "q")
            u2(proj_qk, 1, 1, "k")
            u2(proj_v, 0, 0)
            u2(proj_v, 0, 1)
            alphas(0, pops=(1, 2, 3, 4, 5, 6, 7))
            u2(proj_qk, 0, 2, "q")
            u2(proj_qk, 0, 2, "k")
            u2(proj_qk, 1, 2, "q")
            u2(proj_qk, 1, 2, "k")
            u2(proj_v, 0, 2)
            u2(proj_v, 0, 3)
            u2(proj_v, 1, 0)
            u2(proj_v, 1, 1)
            alphas(2, pops=tuple(range(KT)))
            u2(proj_qk, 0, 3, "q")
            u2(proj_qk, 0, 3, "k")
            u2(proj_qk, 1, 3, "q")
            u2(proj_qk, 1, 3, "k")
            u2(proj_v, 1, 2)
            u2(proj_v, 1, 3)
            u2(avs_qc, 0, 0)
            u2(avs_qc, 0, 1)
            alphas(4, pops=tuple(range(KT)))
            u2(avs_qc, 1, 0)
            u2(avs_qc, 1, 1)
            # head 2 (pair-2 h0) is consumable once pair-2 exps finish, i.e.
            # right as pair 3's stream begins — fill alphas(6)'s spare slots
            u2(avs_qc, 2, 0)
            u2(avs_qc, 2, 1)
            u2(avs_qc, 3, 0)
            u2(avs_qc, 3, 1)
            alphas(6, pops=(2, 3, 4, 5, 6, 7))
            while fill_q:
                fill_q.pop(0)()
            avs_qc(4, 0)
            avs_qc(4, 1)
            avs_qc(5, 0)
            avs_qc(5, 1)
            avs_qc(6, 0)
            avs_qc(6, 1)
            # head 7's first AV halves use pt pairs 0-1 (ready ~4 exps early);
            # only the two closing 2-DR chains depend on the final exp
            avs_qc(7, 0, part=0)
            avs_qc(7, 1, part=0)
            avs_qc(7, 0, part=1)
            avs_qc(7, 1, part=1, last=True)

    nc.compile()
    return nc


_NC_CACHE = {}


def _get_nc(mode):
    if mode not in _NC_CACHE:
        if mode != "fp8":
            raise ValueError(f"unsupported mode {mode}")
        _NC_CACHE[mode] = build_fp8()
    return _NC_CACHE[mode]


def _sbuf_layout_x(xT):
    """[H, S] transposed input -> per-chunk [128, KT*512] SBUF image"""
    x4 = xT.reshape(KT, 128, 2, 512)          # [k, p, sc, s]
    return [np.ascontiguousarray(
        x4[:, :, sc, :].transpose(1, 0, 2).reshape(128, KT * 512)).astype(E4)
        for sc in range(2)]


def _sbuf_layout_w(wT):
    """[H, GH] transposed weight -> [128, KT*GH] SBUF image"""
    w3 = wT.reshape(KT, 128, GH)
    return np.ascontiguousarray(
        w3.transpose(1, 0, 2).reshape(128, KT * GH)).astype(E4)


def _prep_inputs(inputs):
    q = np.asarray(inputs["query"], np.float32)
    k = np.asarray(inputs["key"], np.float32)
    v = np.asarray(inputs["value"], np.float32)
    Wq = np.asarray(inputs["Wq"], np.float32)
    Wk = np.asarray(inputs["Wk"], np.float32)
    Wv = np.asarray(inputs["Wv"], np.float32)
    bq = np.asarray(inputs["bq"], np.float32)
    bk = np.asarray(inputs["bk"], np.float32)
    bv = np.asarray(inputs["bv"], np.float32)

    xq = [_sbuf_layout_x(q[b].T) for b in range(B)]
    xk = [_sbuf_layout_x(k[b].T) for b in range(B)]
    xv = [_sbuf_layout_x(v[b].T) for b in range(B)]
    in_maps = []
    for c in range(NCORES):
        b, g = c // GROUPS, c % GROUPS
        sl = slice(g * GH, (g + 1) * GH)
        bqk = np.stack([(ESC * bq[sl]).reshape(OT, 128).T,
                        (ESC * bk[sl]).reshape(OT, 128).T],
                       1).reshape(128, 2 * OT)
        in_maps.append({
            "xq0": xq[b][0], "xq1": xq[b][1],
            "xk0": xk[b][0], "xk1": xk[b][1],
            "xv0": xv[b][0], "xv1": xv[b][1],
            "wq": _sbuf_layout_w(ESC * Wq[sl, :].T),
            "wk": _sbuf_layout_w(ESC * Wk[sl, :].T),
            "wv": _sbuf_layout_w(ESC * Wv[sl, :].T),
            "bqk": np.ascontiguousarray(bqk, dtype=np.float32),
            "bv": np.ascontiguousarray(ESC * bv[None, sl]).astype(E4),
            "onesd": np.ones((128, 128), E4),
        })
    return in_maps


def run(inputs, mode=MODE, trace=False):
    nc = _get_nc(mode)
    in_maps = _prep_inputs(inputs)
    res = bass_utils.run_bass_kernel_spmd(
        nc, in_maps, core_ids=list(range(NCORES)), trace=trace)

    masks = np.asarray(inputs["masks"], np.float32)
    query = np.asarray(inputs["query"], np.float32)
    out = np.empty((B, S, H), np.float32)
    for c in range(NCORES):
        b, g = c // GROUPS, c % GROUPS
        hid = res.results[c]["hid"].reshape(HL, DH + 1, S)
        hT = hid[:, :DH, :]                      # (HL, DH, S)  (32x scaled)
        se = hid[:, DH, :]                       # (HL, S)
        blk = (hT / (ESC * se[:, None, :])).transpose(2, 0, 1).reshape(S, GH)
        out[b, :, g * GH:(g + 1) * GH] = blk
    out = out * masks[:, :, None] + query
    return out, res


def kernel(**inputs) -> np.ndarray:
    out, _ = run(inputs)
    return out


# revision 64
# speedup vs baseline: 1.1852x; 1.1852x over previous
"""Multi-head attention (ReLU-gated projections) on 8 Trainium2 NeuronCores.

Problem (hardcoded): B=4, S=1024, H=1024, NH=16, DH=64.
  qp = relu(q @ Wq.T + bq); kp, vp likewise
  alpha = softmax(qh @ kh.T / sqrt(DH)) * mask[q]
  out = (alpha @ vh).reshape(B,S,H) + query

Sharding: 8 cores = 4 batches x 2 head-groups (8 heads / 512 hidden cols each).

fp8 design: all matmuls in fp8 e4m3 (TRN2 flavor: with-inf, max finite 240).
Weights pre-scaled by 32 on the host so their N(0, 1/32) values use e4m3's
normal range; the 32x factors ride through the linear pipeline (qp,kp,vp
all carry 32x) and are compensated in the exp scale (1/(8*32^2)) and a
final /32 on the host. exp also subtracts 3.0 (cancels in softmax) to
keep pt under the 240 cap. Projections and AV use MatmulPerfMode.DoubleRow
(K=256 per instruction, 2x PE throughput); alpha matmuls are
output-rate-bound so they stay plain fp8 with the kz zero-padded-K trick.
The AV stationary keeps a ones column (M=65) so row 64 accumulates sumexp
for free; the per-head V slot is padded to 68 bytes so DoubleRow weight
APs stay 4-byte aligned (ISA restriction s3_lw_dual_fp8).

Host pre-arranges x/w into the exact SBUF layouts so every input DMA is
128 partitions x 4KB contiguous. Consts load first so the PE-clock warmup
(and the ACT exp-table preload) start immediately.

Per-core device kernel (transposed "hidden-on-partitions" layout):
  stage 1: qpT[o,s], kpT[o,s] (transposed) and vp[s,o] (normal) projections
           with fused bias+relu, evacuated to fp8.
  stage 2: per head: alphaT[k,q] psum tiles; pt = exp(alpha*sc - 3) in fp8
           written into paired [128, 2048] tiles; AV via DoubleRow with the
           ones column -> unnormalized hidT (64,S) + sumexp (S) per head;
           host divides, applies mask, adds residual.
"""
import sys

sys.path.insert(0, "/opt/trn_rl_repo")

import os
import numpy as np
import ml_dtypes

import concourse.bass as bass
import concourse.tile as tile
from concourse import bacc, mybir
from concourse import bass_utils

B, S, H = 4, 1024, 1024
NH, DH = 16, 64
NCORES = 8
GROUPS = 2          # head-groups (tensor-parallel dim)
HL = NH // GROUPS   # heads per core = 8
GH = H // GROUPS    # hidden cols per core = 512
KT = H // 128       # contraction k-tiles = 8
OT = GH // 128      # output o-tiles per core = 4
SCALE = 1.0 / float(np.sqrt(DH))
ESC = 32.0          # fp8 weight pre-scale (TRN2 fp8e4 = e4m3-with-inf,
                    # max finite 240: keep relu'd projections under ~170)
VW8 = HL * 68       # padded per-head v slot (64 v + 1 ones + 3 pad) = 544

MODE = os.environ.get("BASS_MM_DT", "fp8")

F32 = mybir.dt.float32
BF16 = mybir.dt.bfloat16
FP8 = mybir.dt.float8e4
DR = mybir.MatmulPerfMode.DoubleRow
E4 = ml_dtypes.float8_e4m3   # e4m3 WITH inf (max 240) — matches TRN2 hw


def build_fp8():
    nc = bacc.Bacc("TRN2", target_bir_lowering=False, debug=False,
                   num_devices=NCORES)

    # x/w arrive pre-arranged in SBUF layout: [128, KT*512] per s-chunk
    x_d = {(w, sc): nc.dram_tensor(f"x{w}{sc}", [128, KT * 512], FP8,
                                   kind="ExternalInput").ap()
           for w in "qkv" for sc in range(2)}
    w_d = {w: nc.dram_tensor(f"w{w}", [128, KT * GH], FP8,
                             kind="ExternalInput").ap()
           for w in "qkv"}
    bqk_d = nc.dram_tensor("bqk", [128, 2 * OT], F32, kind="ExternalInput").ap()
    bv_d = nc.dram_tensor("bv", [1, GH], FP8, kind="ExternalInput").ap()
    ones_d = nc.dram_tensor("onesd", [128, 128], FP8,
                            kind="ExternalInput").ap()
    hid_d = nc.dram_tensor("hid", [HL * (DH + 1), S], F32,
                           kind="ExternalOutput").ap()

    EXP_SCALE = SCALE / (ESC * ESC)
    EXP_BIAS = -3.0   # pt = exp(alpha/8 - 3): keeps exp under e4m3 max 240;
                      # cancels in hid/sumexp

    with tile.TileContext(nc) as tc:
        with tc.tile_pool(name="sb", bufs=1) as sb, \
             tc.tile_pool(name="ps", bufs=1, space="PSUM") as ps:

            # ---- persistent tiles ----
            wq_t = sb.tile([128, KT * GH], FP8, tag="wq", name="wq")
            wk_t = sb.tile([128, KT * GH], FP8, tag="wk", name="wk")
            wv_t = sb.tile([128, KT * GH], FP8, tag="wv", name="wv")
            qp_t = [sb.tile([128, S], FP8, tag=f"qp{t}", name=f"qp{t}")
                    for t in range(OT)]
            kz_t = [[sb.tile([128, S], FP8, tag=f"kz{t}{h}",
                             name=f"kz{t}{h}") for h in range(2)]
                    for t in range(OT)]
            kz_zeroed = set()
            vp_t = sb.tile([128, KT * VW8], FP8, tag="vp", name="vp")
            bqk_t = sb.tile([128, 2 * OT], F32, tag="bqk", name="bqk")
            bv_t = sb.tile([1, GH], FP8, tag="bv", name="bv")
            ones_t = sb.tile([1, 128], FP8, tag="ones", name="ones")
            ones64_t = sb.tile([128, KT * HL], FP8, tag="ones64",
                               name="ones64")
            expb_t = sb.tile([128, 1], F32, tag="expb", name="expb")
            nc.vector.memset(expb_t[:], EXP_BIAS)

            # ---- warmup from memset tiles: no DMA dependency, so the PE
            #      clock ramp and the ACT exp-table preload start at ~1us ----
            wstat = sb.tile([1, 128], FP8, tag="wstat", name="wstat")
            wmov = sb.tile([1, 512], FP8, tag="wmov", name="wmov")
            nc.vector.memset(wstat[:], 1.0)
            nc.vector.memset(wmov[:], 1.0)
            warm = ps.tile([65, 512], F32, tag="av", bufs=2, name="warm")
            for i in range(12):
                nc.tensor.matmul(warm[:], wstat[:, 0:65], wmov[:],
                                 start=True, stop=True)
            dummy_exp = sb.tile([1, 8], F32, tag="dummy_exp", name="dummy_exp")
            nc.scalar.activation(dummy_exp[:], wmov[0:1, 0:8],
                                 mybir.ActivationFunctionType.Exp, scale=1.0)

            # ---- loads: whole tiles (4KB contiguous runs), three rings in
            #      parallel, priority-ordered by first use ----
            x_t = {}
            rings = [nc.sync, nc.scalar, nc.gpsimd]
            ring_i = [0]

            def x_ld(which, sc, eng):
                t = sb.tile([128, KT * 512], FP8, tag=f"x{which}{sc}",
                            name=f"x{which}_{sc}")
                x_t[(which, sc)] = t
                eng.dma_start(t[:], x_d[(which, sc)])

            def x3(which, sc):
                return x_t[(which, sc)][:].rearrange("p (k s) -> p k s", s=512)

            nc.sync.dma_start(bqk_t[:], bqk_d)
            x_ld("q", 0, nc.sync)
            x_ld("k", 0, nc.scalar)
            nc.gpsimd.dma_start(wq_t[:], w_d["q"])
            nc.gpsimd.dma_start(wk_t[:], w_d["k"])
            x_ld("k", 1, nc.sync)
            x_ld("q", 1, nc.scalar)
            nc.gpsimd.dma_start(bv_t[:], bv_d)
            nc.gpsimd.dma_start(ones_t[:], ones_d[0:1, :])
            nc.sync.dma_start(ones64_t[:], ones_d[:, 0:KT * HL])
            x_ld("v", 0, nc.sync)
            x_ld("v", 1, nc.scalar)
            nc.gpsimd.dma_start(wv_t[:], w_d["v"])

            # ones column of the AV stationary
            v4 = vp_t[:].rearrange("p (k n c) -> p k n c", n=HL, c=68)
            nc.vector.tensor_copy(
                v4[:, :, :, DH:DH + 1],
                ones64_t[:].rearrange("p (k n one) -> p k n one", n=HL, one=1))

            pp_live = {}

            def proj_qk(sc, ot, which, part=None):
                """one o-tile, one s-chunk of the transposed q/k projection;
                part 0/1 emit half the DR chain each (fill-unit sizing),
                part None emits the whole group."""
                w_t = wq_t if which == "q" else wk_t
                w3 = w_t[:].rearrange("p (k o) -> p k o", o=GH)
                xv_ = x3(which, sc)
                if part != 1:
                    pp_live[(sc, ot, which)] = ps.tile(
                        [128, 1024], F32, tag="alpha", bufs=3,
                        name=f"pp{which}_{sc}_{ot}")
                pp = pp_live[(sc, ot, which)]
                kps = range(KT // 2) if part is None else (
                    range(2) if part == 0 else range(2, 4))
                for kp in kps:
                    nc.tensor.matmul(
                        pp[:, 0:512],
                        w3[:, 2 * kp:2 * kp + 2, ot * 128:(ot + 1) * 128],
                        xv_[:, 2 * kp:2 * kp + 2, :],
                        start=(kp == 0), stop=(kp == KT // 2 - 1),
                        perf_mode=DR)
                if part == 0:
                    return
                wi = 0 if which == "q" else 1
                bias = bqk_t[:, wi * OT + ot:wi * OT + ot + 1]
                ssl = slice(sc * 512, (sc + 1) * 512)
                if which == "q":
                    if (sc, ot) == (1, 0):
                        # critical-path evac before the first alpha: ACT is
                        # idle pre-stream while the DVE queue is backed up
                        nc.scalar.activation(
                            qp_t[ot][:, ssl], pp[:, 0:512],
                            mybir.ActivationFunctionType.Relu,
                            bias=bias, scale=1.0)
                    else:
                        nc.vector.tensor_scalar(
                            qp_t[ot][:, ssl], pp[:, 0:512], bias, 0.0,
                            mybir.AluOpType.add, mybir.AluOpType.max)
                else:
                    for h in range(2):
                        pr = slice(h * 64, h * 64 + 64)
                        nc.vector.tensor_scalar(
                            kz_t[ot][h][pr, ssl], pp[pr, 0:512], bias[pr, :],
                            0.0, mybir.AluOpType.add, mybir.AluOpType.max)
                pp_live.pop((sc, ot, which))

            def proj_v(sc, j, part=None):
                """one s-tile (128 rows of vp) within chunk sc"""
                st = sc * 4 + j
                wv3 = wv_t[:].rearrange("p (k o) -> p k o", o=GH)
                xv_ = x3("v", sc)
                if part != 1:
                    pp_live[("v", st)] = ps.tile([128, 1024], F32,
                                                 tag="alpha", bufs=3,
                                                 name=f"ppv_{st}")
                    nc.tensor.matmul(pp_live[("v", st)][:, 0:512],
                                     ones_t[:], bv_t[:],
                                     start=True, stop=False)
                pp = pp_live[("v", st)]
                kps = range(KT // 2) if part is None else (
                    range(2) if part == 0 else range(2, 4))
                for kp in kps:
                    nc.tensor.matmul(
                        pp[:, 0:512],
                        xv_[:, 2 * kp:2 * kp + 2, j * 128:(j + 1) * 128],
                        wv3[:, 2 * kp:2 * kp + 2, :],
                        start=False, stop=(kp == KT // 2 - 1),
                        perf_mode=DR)
                if part == 0:
                    return
                v3 = vp_t[:, st * VW8:(st + 1) * VW8].rearrange(
                    "p (n c) -> p n c", c=68)
                p3 = pp[:, 0:512].rearrange("p (n c) -> p n c", c=DH)
                nc.vector.tensor_scalar(
                    v3[:, :, 0:DH], p3, 0.0, None, mybir.AluOpType.max)
                pp_live.pop(("v", st))

            pt_all = {}
            fill_q = []

            def alphas(n0, pops=(1, 3, 5, 7)):
                """alpha + exp for head pair (n0, n0+1), head-major so each
                head's pt tiles complete early and its AV can start while the
                other head's exps still stream.  pt tiles are paired
                [128, 2048] (two k-tiles) so AV consumes them via DoubleRow.
                Between apt tiles, pop small PE work units from fill_q so the
                PE's ACT-rate-limited stall time does useful work."""
                t = n0 // 2
                if t not in kz_zeroed:
                    kz_zeroed.add(t)
                    nc.gpsimd.memset(kz_t[t][0][64:128, :], 0.0)
                    nc.gpsimd.memset(kz_t[t][1][0:64, :], 0.0)
                for h in range(2):
                    pts = []
                    cur = None
                    for k in range(KT):
                        apt = ps.tile([128, 1024], F32, tag="alpha", bufs=3,
                                      name=f"alp_{n0 + h}_{k}")
                        for qc in range(2):
                            nc.tensor.matmul(
                                apt[:, qc * 512:(qc + 1) * 512],
                                kz_t[t][h][:, k * 128:(k + 1) * 128],
                                qp_t[t][:, qc * 512:(qc + 1) * 512],
                                start=True, stop=True)
                        half = k % 2
                        if half == 0:
                            cur = sb.tile([128, 2048], FP8, tag="pt",
                                          bufs=32, name=f"pt_{n0 + h}_{k}")
                            pts.append(cur)
                        nc.scalar.activation(
                            cur[:, half * 1024:(half + 1) * 1024], apt[:],
                            mybir.ActivationFunctionType.Exp, scale=EXP_SCALE,
                            bias=expb_t[:])
                        if k in pops and fill_q:
                            fill_q.pop(0)()
                    pt_all[n0 + h] = pts

            hid_tiles = {}
            av_live = {}

            def avs_qc(n, qc, last=False, part=None):
                pts = pt_all[n]
                if qc == 0 and part != 1:
                    hid_tiles[n] = sb.tile([DH + 1, S], F32, tag="hid",
                                           bufs=3, name=f"hid_{n}")
                hid_t = hid_tiles[n]
                if part != 1:
                    av_live[(n, qc)] = ps.tile([DH + 1, 512], F32, tag="av",
                                               bufs=2, name=f"av_{n}_{qc}")
                av = av_live[(n, qc)]
                kps = range(KT // 2) if part is None else (
                    range(2) if part == 0 else range(2, 4))
                for kp in kps:
                    nc.tensor.matmul(
                        av[:],
                        v4[:, 2 * kp:2 * kp + 2, n, 0:DH + 1],
                        pts[kp][:].rearrange(
                            "p (k s) -> p k s",
                            s=1024)[:, :, qc * 512:(qc + 1) * 512],
                        start=(kp == 0), stop=(kp == KT // 2 - 1),
                        perf_mode=DR)
                if part == 0:
                    return
                av_live.pop((n, qc))
                if last:
                    # ACT is idle after its final exp — use it so the two
                    # tail evacuations run on different engines
                    nc.scalar.copy(
                        hid_t[:, qc * 512:(qc + 1) * 512], av[:])
                else:
                    nc.vector.tensor_copy(
                        hid_t[:, qc * 512:(qc + 1) * 512], av[:])
                # never the scalar ring: a DMA descriptor op there would
                # steal ~0.8us from the ACT exp stream
                eng = nc.sync if ring_i[0] % 2 == 0 else nc.gpsimd
                ring_i[0] += 1
                eng.dma_start(
                    hid_d[n * (DH + 1):(n + 1) * (DH + 1),
                          qc * 512:(qc + 1) * 512],
                    hid_t[:, qc * 512:(qc + 1) * 512])
                if qc == 1:
                    pt_all.pop(n)
                    hid_tiles.pop(n)

            # ---- emission schedule: the exp stream (ACT) is the metronome.
            #      All other PE work is queued as fill units popped between
            #      alpha tiles, so the PE's ACT-limited stalls do the
            #      projections and AV chunks. Queue order respects deps:
            #      o-tile t's projections drain inside alphas(2(t-1)). ----
            def u2(f, *a):
                fill_q.append(lambda: f(*a, part=0))
                fill_q.append(lambda: f(*a, part=1))

            # alphas(0) k-tiles 0-3 read only the sc0 half of kz o-tile 0, so
            # the sc1 k-projection rides the fill queue (its parts pop at
            # k=1,2 — done before the k=4 alpha tile needs them)
            proj_qk(0, 0, "q")
            # o-tile-1's sc0 q-projection needs only first-wave DMA data
            # (xq0+wq): run it in the PE idle slot while wk/xq1 land
            proj_qk(0, 1, "q")
            proj_qk(0, 0, "k")
            proj_qk(1, 0, "q")
            u2(proj_qk, 1, 0, "k")
            u2(proj_qk, 0, 1, "k")
            u2(proj_qk, 1, 1, "q")
            u2(proj_qk, 1, 1, "k")
            u2(proj_v, 0, 0)
            u2(proj_v, 0, 1)
            alphas(0, pops=(1, 2, 3, 4, 5, 6, 7))
            u2(proj_qk, 0, 2, "q")
            u2(proj_qk, 0, 2, "k")
            u2(proj_qk, 1, 2, "q")
            u2(proj_qk, 1, 2, "k")
            u2(proj_v, 0, 2)
            u2(proj_v, 0, 3)
            u2(proj_v, 1, 0)
            u2(proj_v, 1, 1)
            alphas(2, pops=tuple(range(KT)))
            u2(proj_qk, 0, 3, "q")
            u2(proj_qk, 0, 3, "k")
            u2(proj_qk, 1, 3, "q")
            u2(proj_qk, 1, 3, "k")
            u2(proj_v, 1, 2)
            u2(proj_v, 1, 3)
            u2(avs_qc, 0, 0)
            u2(avs_qc, 0, 1)
            alphas(4, pops=tuple(range(KT)))
            u2(avs_qc, 1, 0)
            u2(avs_qc, 1, 1)
            # head 2 (pair-2 h0) is consumable once pair-2 exps finish, i.e.
            # right as pair 3's stream begins — fill alphas(6)'s spare slots
            u2(avs_qc, 2, 0)
            u2(avs_qc, 2, 1)
            u2(avs_qc, 3, 0)
            u2(avs_qc, 3, 1)
            alphas(6, pops=(2, 3, 4, 5, 6, 7))
            while fill_q:
                fill_q.pop(0)()
            avs_qc(4, 0)
            avs_qc(4, 1)
            avs_qc(5, 0)
            avs_qc(5, 1)
            avs_qc(6, 0)
            avs_qc(6, 1)
            # head 7's first AV halves use pt pairs 0-1 (ready ~4 exps early);
            # only the two closing 2-DR chains depend on the final exp
            avs_qc(7, 0, part=0)
            avs_qc(7, 1, part=0)
            avs_qc(7, 0, part=1)
            avs_qc(7, 1, part=1, last=True)

    nc.compile()
    return nc


_NC_CACHE = {}


def _get_nc(mode):
    if mode not in _NC_CACHE:
        if mode != "fp8":
            raise ValueError(f"unsupported mode {mode}")
        _NC_CACHE[mode] = build_fp8()
    return _NC_CACHE[mode]


def _sbuf_layout_x(xT):
    """[H, S] transposed input -> per-chunk [128, KT*512] SBUF image"""
    x4 = xT.reshape(KT, 128, 2, 512)          # [k, p, sc, s]
    return [np.ascontiguousarray(
        x4[:, :, sc, :].transpose(1, 0, 2).reshape(128, KT * 512)).astype(E4)
        for sc in range(2)]


def _sbuf_layout_w(wT):
    """[H, GH] transposed weight -> [128, KT*GH] SBUF image"""
    w3 = wT.reshape(KT, 128, GH)
    return np.ascontiguousarray(
        w3.transpose(1, 0, 2).reshape(128, KT * GH)).astype(E4)


def _prep_inputs(inputs):
    q = np.asarray(inputs["query"], np.float32)
    k = np.asarray(inputs["key"], np.float32)
    v = np.asarray(inputs["value"], np.float32)
    Wq = np.asarray(inputs["Wq"], np.float32)
    Wk = np.asarray(inputs["Wk"], np.float32)
    Wv = np.asarray(inputs["Wv"], np.float32)
    bq = np.asarray(inputs["bq"], np.float32)
    bk = np.asarray(inputs["bk"], np.float32)
    bv = np.asarray(inputs["bv"], np.float32)

    xq = [_sbuf_layout_x(q[b].T) for b in range(B)]
    xk = [_sbuf_layout_x(k[b].T) for b in range(B)]
    xv = [_sbuf_layout_x(v[b].T) for b in range(B)]
    in_maps = []
    for c in range(NCORES):
        b, g = c // GROUPS, c % GROUPS
        sl = slice(g * GH, (g + 1) * GH)
        bqk = np.stack([(ESC * bq[sl]).reshape(OT, 128).T,
                        (ESC * bk[sl]).reshape(OT, 128).T],
                       1).reshape(128, 2 * OT)
        in_maps.append({
            "xq0": xq[b][0], "xq1": xq[b][1],
            "xk0": xk[b][0], "xk1": xk[b][1],
            "xv0": xv[b][0], "xv1": xv[b][1],
            "wq": _sbuf_layout_w(ESC * Wq[sl, :].T),
            "wk": _sbuf_layout_w(ESC * Wk[sl, :].T),
            "wv": _sbuf_layout_w(ESC * Wv[sl, :].T),
            "bqk": np.ascontiguousarray(bqk, dtype=np.float32),
            "bv": np.ascontiguousarray(ESC * bv[None, sl]).astype(E4),
            "onesd": np.ones((128, 128), E4),
        })
    return in_maps


def run(inputs, mode=MODE, trace=False):
    nc = _get_nc(mode)
    in_maps = _prep_inputs(inputs)
    res = bass_utils.run_bass_kernel_spmd(
        nc, in_maps, core_ids=list(range(NCORES)), trace=trace)

    masks = np.asarray(inputs["masks"], np.float32)
    query = np.asarray(inputs["query"], np.float32)
    out = np.empty((B, S, H), np.float32)
    for c in range(NCORES):
        b, g = c // GROUPS, c % GROUPS
        hid = res.results[c]["hid"].reshape(HL, DH + 1, S)
        hT = hid[:, :DH, :]                      # (HL, DH, S)  (32x scaled)
        se = hid[:, DH, :]                       # (HL, S)
        blk = (hT / (ESC * se[:, None, :])).transpose(2, 0, 1).reshape(S, GH)
        out[b, :, g * GH:(g + 1) * GH] = blk
    out = out * masks[:, :, None] + query
    return out, res


def kernel(**inputs) -> np.ndarray:
    out, _ = run(inputs)
    return out
